# revision 2
# baseline (speedup 1.0000x reference)
"""PSNR-conv kernel for TRN2 (8 NeuronCores, SPMD) — fp8 DoubleRow version.

Math (per channel c, per 16x16 window):
    mse_c*256 = sum w*x^2 - 2 sum k_c*w*x + sum k_c^2*w
Centered at y = x - 128 (halves fp8 quantization error):
    mse_c*256 = sum w*y^2 + sum d_c*y + const_c,
      d_c = 2w*(128 - k_c),  const_c = 16384*sum(w) - 256*sum(k_c w) + sum(k_c^2 w)
Both convs are fused into ONE fp8e4 DoubleRow matmul per kernel column dj:
contraction K=256 = [y^2-plane band | y-plane band], each a 16-tap Toeplitz
band over 128 input rows. PSUM accumulates (sum w y^2 + sum d y)/64 over the
16 dj. ScalarE Ln(+const/64 bias) and VectorE combine produce the output.

Scales: lhsT = [fp8(2w) | fp8(d/8)], rhs = [fp8(y^2/128) | fp8(y/8)] so both
k-tiles contribute at 1/64 scale. On-device prep from uint8 pixels:
ACT Square(x/sqrt(128) - sqrt(128)) -> y^2/128 (fp8), DVE x*(1/8) - 16 -> y/8
(fp8). Input is uint8 (x rounded on host: exact integer centering, error
comparable to bf16 at half the DMA bytes — input DMA exposed ~25 us of
critical path at bf16, ~0 at uint8). Output is bf16 (psnr in dB, rel err
2^-9 ~ 2e-3 of a ~30 dB value; halves output DMA), cast to fp32 on host.

DoubleRow ISA restriction: the k-tile-pair stride (step_elem[2]) must be a
multiple of 16 elements on BOTH Ldweights and Matmult sides — hence weight
columns padded to MBP=128 and per-channel feature width padded to CIP=528.

Sharding: 2x4 grid of overlapping strips, SPMD identical instruction stream,
9 row blocks of 113 output rows x 510 output cols per core.
"""

import sys

if "/opt/trn_rl_repo" not in sys.path:
    sys.path.insert(0, "/opt/trn_rl_repo")

import numpy as np
import ml_dtypes

PIXEL_MAX = 255.0
C, Hk, Wk = 4, 16, 16
H = W = 2048
HO = WO = H - Hk + 1          # 2033
MB = 113                      # output rows per block (128 - 15)
MBP = 128                     # padded weight columns (stride mult of 16)
KP = 128                      # contraction rows per k-tile
NRB = 9                       # row blocks per core; 9*113 = 1017 rows
OUT_ROWS = NRB * MB           # 1017
NCOL = 510                    # output cols per core
IN_COLS = NCOL + Hk - 1       # 525
CIP = 528                     # padded feature width (stride mult of 16)
IN_ROWS = OUT_ROWS + Hk - 1   # 1032
ROW_STARTS = [0, HO - OUT_ROWS]                    # [0, 1016]
COL_STARTS = [0, 507, 1015, WO - NCOL]             # [0, 507, 1015, 1523]
N_CORES = 8

A_CONST = 20.0 * np.log10(PIXEL_MAX)
B_CONST = 10.0 / (4.0 * np.log(10.0))
# ln(mse) = Ln(psum + const/64) - ln 4  =>  fold 4*B*ln4 into the A constant
A_EFF = A_CONST + 4.0 * B_CONST * np.log(4.0)
SQ_SCALE = float(1.0 / np.sqrt(128.0))
SQ_BIAS = float(-np.sqrt(128.0))

F8 = ml_dtypes.float8_e4m3
NCOL_MM = NCOL  # matmul moving width; narrowed only by timing experiments
SKIP_SQUARE = False  # timing experiments: drop pipeline stages (wrong results)
SKIP_Y8 = False
SKIP_LN = False
SKIP_TAIL = False
SKIP_INDMA = False


def _build_nc(biases, reps=1):
    """biases: 4 floats, const_c/64 for the Ln activation.

    reps > 1 repeats the whole body inside one NEFF (idempotent — same
    output written each rep); used only for timing, where the marginal
    cost per extra rep isolates pure on-device body time."""
    import concourse.bacc as bacc
    import concourse.mybir as mybir
    from concourse.tile import TileContext

    f32 = mybir.dt.float32
    bf16 = mybir.dt.bfloat16
    u8 = mybir.dt.uint8
    f8 = mybir.dt.float8e4
    Ln = mybir.ActivationFunctionType.Ln
    Square = mybir.ActivationFunctionType.Square
    mult = mybir.AluOpType.mult
    add = mybir.AluOpType.add
    DR = mybir.MatmulPerfMode.DoubleRow

    nc = bacc.Bacc("TRN2", target_bir_lowering=False, debug=False)

    # Register const APs for activation biases (ACT bias-as-float looks
    # immediates up in const_aps).
    for v in list(biases) + [SQ_BIAS]:
        v = float(v)
        t = nc.alloc_sbuf_tensor(f"const-float32-{v}", [128, 1], f32)
        nc.gpsimd.memset(t.ap(), v)
        nc.const_aps.aps[(f32, v)] = t.ap()
    nc.all_engine_barrier()

    # row-major uint8 input (host pre-rounded + transposed to
    # [rows, C*IN_COLS]): halves DMA bytes vs bf16, and each block's DMA is
    # a 2D rectangle, split in two for queue parallelism
    xs = nc.dram_tensor("xs", [IN_ROWS, C * IN_COLS], u8,
                        kind="ExternalInput")
    # DoubleRow Toeplitz weights, k-major so the DMA is contiguous per row:
    # [KP, C, Hk, 2, MBP] flattened to [KP, C*Hk*2*MBP]
    w8 = nc.dram_tensor("w8", [KP, C * Hk * 2 * MBP], f8, kind="ExternalInput")
    out = nc.dram_tensor("out", [OUT_ROWS, NCOL], bf16, kind="ExternalOutput")

    with TileContext(nc) as tc:
        with (
            tc.tile_pool(name="wpool", bufs=1) as wpool,
            tc.tile_pool(name="xpool", bufs=4) as xpool,
            tc.tile_pool(name="fpool", bufs=4) as fpool,
            tc.tile_pool(name="lnpool", bufs=2) as lnpool,
            tc.tile_pool(name="opool", bufs=2) as opool,
            tc.tile_pool(name="pspool", bufs=2, space="PSUM") as pspool,
        ):
            wt = wpool.tile([KP, C * Hk * 2 * MBP], f8)
            nc.sync.dma_start(wt[:], w8[:])
            wv = wt[:].rearrange(
                "k (c d two m) -> k c d two m", c=C, d=Hk, two=2
            )

            def prep(i):
                """DMA + fp8 feature build for block i. Emitted one block
                ahead of its consumer so Square/y8 for block i+1 precede
                block i's Ln ops in the ACT/DVE queues — otherwise the PE
                stalls ~3 us per block waiting on the ACT engine to drain
                Lns before the next Square."""
                row0 = MB * (i % NRB)
                xt = xpool.tile([KP, C * IN_COLS], u8, tag="xt")
                if not SKIP_INDMA:
                    nc.sync.dma_start(
                        xt[0:KP // 2, :], xs[row0:row0 + KP // 2, :])
                    nc.sync.dma_start(
                        xt[KP // 2:KP, :],
                        xs[row0 + KP // 2:row0 + KP, :])
                else:
                    nc.sync.dma_start(xt[:, 0:4], xs[row0:row0 + KP, 0:4])
                xv = xt[:].rearrange("k (c w) -> k c w", c=C)
                # fp8 feature tile [KP, C, 2, CIP]: per channel
                # [ y^2/128 | y/8 ], k-tile-pair stride CIP = 528
                ft = fpool.tile([KP, C * 2 * CIP], f8, tag="ft")
                fw = ft[:].rearrange("k (c two w) -> k c two w", c=C, two=2)
                nc.scalar.activation(
                    fw[:, :, 0, 0:(IN_COLS if not SKIP_SQUARE else 4)],
                    xv if not SKIP_SQUARE else xv[:, :, 0:4], Square,
                    bias=SQ_BIAS, scale=SQ_SCALE,
                )
                nc.vector.tensor_scalar(
                    fw[:, :, 1, 0:(IN_COLS if not SKIP_Y8 else 4)],
                    xv if not SKIP_Y8 else xv[:, :, 0:4],
                    0.125, -16.0, mult, add
                )
                return fw

            # Row blocks are processed in PAIRS: the two matmuls of a pair
            # share one weight load (the redundant second InstLdweights is
            # stripped post-compile — dual-fp8 Ldweights does not overlap
            # the moving stream, costing ~113 cy per load otherwise).
            n_iter = NRB * reps
            fts = {}

            def ensure_prep(i):
                if i < n_iter and i not in fts:
                    fts[i] = prep(i)

            def tail(rb, lns):
                row0 = MB * rb
                ob = opool.tile([MB, NCOL], bf16, tag="ob")
                if not SKIP_TAIL:
                    s01 = opool.tile([MB, NCOL], f32, tag="s01")
                    s23 = opool.tile([MB, NCOL], f32, tag="s23")
                    nc.vector.tensor_add(s01[:], lns[0][:], lns[1][:])
                    nc.vector.tensor_add(s23[:], lns[2][:], lns[3][:])
                    t = opool.tile([MB, NCOL], f32, tag="t")
                    nc.vector.tensor_add(t[:], s01[:], s23[:])
                    nc.vector.tensor_scalar(
                        ob[:], t[:], -B_CONST, A_EFF, mult, add
                    )
                else:
                    nc.vector.tensor_scalar(
                        ob[:, 0:4], lns[0][:, 0:4], -B_CONST, A_EFF,
                        mult, add
                    )
                nc.sync.dma_start(out[row0:row0 + MB, :], ob[:])

            ensure_prep(0)
            ensure_prep(1)
            for pa in range(0, n_iter, 2):
                pb = pa + 1 if pa + 1 < n_iter else None
                ensure_prep(pa + 2)
                ensure_prep(pa + 3)
                fwa = fts.pop(pa)
                fwb = fts.pop(pb) if pb is not None else None

                lnsa, lnsb = [], []
                for c in range(C):
                    psa = pspool.tile([MB, NCOL], f32, tag=f"ps{c}",
                                      name=f"ps{c}")
                    if fwb is not None:
                        psb = pspool.tile([MB, NCOL], f32, tag=f"ps{c}",
                                          name=f"ps{c}b")
                    for dj in range(Hk):
                        nc.tensor.matmul(
                            psa[:, 0:NCOL_MM],
                            wv[:, c, dj, :, 0:MB],
                            fwa[:, c, :, dj:dj + NCOL_MM],
                            start=(dj == 0), stop=(dj == Hk - 1),
                            perf_mode=DR,
                        )
                        if fwb is not None:
                            nc.tensor.matmul(
                                psb[:, 0:NCOL_MM],
                                wv[:, c, dj, :, 0:MB],
                                fwb[:, c, :, dj:dj + NCOL_MM],
                                start=(dj == 0), stop=(dj == Hk - 1),
                                perf_mode=DR,
                            )
                    lnca = lnpool.tile([MB, NCOL], f32, tag=f"ln{c}",
                                       name=f"ln{c}")
                    if not SKIP_LN:
                        nc.scalar.activation(
                            lnca[:], psa[:], Ln, bias=float(biases[c]),
                            scale=1.0
                        )
                    else:
                        nc.scalar.activation(
                            lnca[:, 0:4], psa[:, 0:4], Ln,
                            bias=float(biases[c]), scale=1.0
                        )
                    lnsa.append(lnca)
                    if fwb is not None:
                        lncb = lnpool.tile([MB, NCOL], f32, tag=f"ln{c}",
                                           name=f"ln{c}b")
                        if not SKIP_LN:
                            nc.scalar.activation(
                                lncb[:], psb[:], Ln, bias=float(biases[c]),
                                scale=1.0
                            )
                        else:
                            nc.scalar.activation(
                                lncb[:, 0:4], psb[:, 0:4], Ln,
                                bias=float(biases[c]), scale=1.0
                            )
                        lnsb.append(lncb)

                tail(pa % NRB, lnsa)
                if fwb is not None:
                    tail(pb % NRB, lnsb)

    nc.compile()
    _dedup_ldweights(nc)
    return nc


def _dedup_ldweights(nc):
    """Drop an InstLdweights whose weights AP equals the still-loaded one
    (same AP, no intervening PE instruction other than Matmult) and which
    carries no semaphore waits/updates. The simulator ignores Ldweights
    entirely (matmult reads weights from its own AP), so simulation
    semantics are unchanged; on hardware the matmult reuses the loaded
    stationary."""
    import concourse.mybir as mybir

    removed = 0
    for blk in nc.m.functions[0].blocks:
        insts = list(blk.instructions)
        out = []
        blk_removed = 0
        last_key = None
        for inst in insts:
            tn = type(inst).__name__
            if tn == "InstLdweights":
                si = inst.sync_info
                clean = si is None or (not si.on_wait and not si.on_update)
                key = str(inst.ins[0])
                if clean and key == last_key:
                    blk_removed += 1
                    continue
                last_key = key
            elif tn != "InstMatmult" and getattr(
                    inst, "engine", None) == mybir.EngineType.PE:
                last_key = None
            out.append(inst)
        if blk_removed:
            blk.instructions = out
            removed += blk_removed
    return removed


def _prep_weights(kernel):
    """Host prep: fp8 DoubleRow Toeplitz weights + Ln biases."""
    k = np.asarray(kernel)[0].astype(np.float64)        # (4, 16, 16)
    alpha = k[3] / PIXEL_MAX
    w = (1.0 - alpha) ** 2                              # (16, 16)
    d = 2.0 * w[None] * (128.0 - k)                     # (4, 16, 16)

    wq2 = np.asarray(2.0 * w, F8).astype(np.float64)    # quantized 2w taps
    dq8 = np.asarray(d / 8.0, F8).astype(np.float64)    # quantized d/8 taps

    # W8[k, c, dj, t, m]: t=0 band of 2w[:, dj], t=1 band of d_c[:, dj]/8
    w8 = np.zeros((KP, C, Hk, 2, MBP), np.float64)
    idx = np.arange(MB)
    for dj in range(Hk):
        for di in range(Hk):
            w8[idx + di, :, dj, 0, idx] = wq2[di, dj]
            for c in range(C):
                w8[idx + di, c, dj, 1, idx] = dq8[c, di, dj]
    w8 = np.ascontiguousarray(w8.reshape(KP, C * Hk * 2 * MBP)).astype(F8)

    const = (16384.0 * w.sum() - 256.0 * (k * w[None]).sum(axis=(1, 2))
             + (k * k * w[None]).sum(axis=(1, 2)))      # (4,)
    biases = const / 64.0
    return w8, biases


def make_in_maps(x, kernel):
    """Host-side input prep shared by kernel() and the timing harness:
    per-core strips as uint8 [IN_ROWS, C*IN_COLS] plus the fp8 weights."""
    w8, biases = _prep_weights(kernel)
    # round once on the full image (cheaper than per-strip; strips overlap)
    xq = np.rint(np.asarray(np.asarray(x)[0], np.float32)).astype(np.uint8)
    in_maps = []
    for r in range(2):
        for cc in range(4):
            r0, c0 = ROW_STARTS[r], COL_STARTS[cc]
            strip = xq[:, r0:r0 + IN_ROWS, c0:c0 + IN_COLS]
            in_maps.append({
                "xs": np.ascontiguousarray(
                    strip.transpose(1, 0, 2).reshape(IN_ROWS, C * IN_COLS)
                ),
                "w8": w8,
            })
    return in_maps, biases


def kernel(x, kernel):
    from concourse.bass_utils import run_bass_kernel_spmd

    in_maps, biases = make_in_maps(x, kernel)
    nc = _build_nc(biases)

    res = run_bass_kernel_spmd(nc, in_maps, core_ids=list(range(N_CORES)))

    full = np.empty((HO, WO), np.float32)
    for r in range(2):
        for cc in range(4):
            core = r * 4 + cc
            r0, c0 = ROW_STARTS[r], COL_STARTS[cc]
            full[r0:r0 + OUT_ROWS, c0:c0 + NCOL] = (
                res.results[core]["out"].astype(np.float32))
    return full
